# revision 1
# baseline (speedup 1.0000x reference)
"""Trainium2 Bass kernel for DomainCalibratedLoss.

loss_i = lse_j(logw[d_i, j] + x[i, j]) - (logw[d_i, t_i] + x[i, t_i])
out    = sum_i(loss_i) / N

Strategy (data-parallel over 8 cores, ~62500 rows each):
  * rows are laid out [P=128 partitions, R slots/partition, 200 classes] per
    "supertile"; row = base + p*R + r so each partition reads contiguous DRAM.
  * per-row bias rows logw[d_i, :] are delivered by a PE matmul:
    host-built one-hot lhsT [32, P] x const rhs [32, 400] (hi/lo bf16 split of
    logw for f32-grade accuracy) -> PSUM [P, 2*200] for each slot pair.
  * DVE adds X + bias (PSUM) -> scores (bf16).
  * ACT exp(scores) -> E (bf16); DVE 3D tensor_reduce gives per-slot sums S.
  * the target one-hot mask arrives pre-baked from the host as fp8 (this
    walrus cannot compile GPSIMD tensor ops or tensor_tensor_reduce, and it
    caps every engine instruction at ONE sync wait -- see
    _prune_redundant_waits); DVE multiplies scores*mask and ACT reduces the
    product with Copy+accum into per-supertile V partials.
  * tail: ACT Ln over all S values, DVE reduces, per-core [128, 2]
    partial sums (sum ln S | sum target-scores) -> host combines.

No max-subtraction is needed: scores are in [-6.5, 13.5] for this data so
exp stays comfortably inside f32 range.
"""

import math
from contextlib import ExitStack

import numpy as np

import concourse.bass as bass
import concourse.tile as tile
from concourse import mybir
from concourse.tile_rust import add_dep_helper
from concourse.bass_utils import run_bass_kernel_spmd

F32 = mybir.dt.float32
BF16 = mybir.dt.bfloat16
FP8 = mybir.dt.float8e4
BF16_NP = mybir.dt.np(BF16)
FP8_NP = mybir.dt.np(FP8)
MASK_DT = FP8
MASK_DT_NP = FP8_NP

N_TOTAL = 500000
N_CORES = 8
N_PER = N_TOTAL // N_CORES
C = 200  # classes
D = 8  # domains
IGNORE = 255


def _plan(n_rows):
    """Split n_rows into supertiles (base, P, R): row = base + p*R + r."""
    plan = []
    base = 0
    while n_rows - base >= 128 * 16:
        plan.append((base, 128, 16))
        base += 128 * 16
    left = n_rows - base
    if left:
        r = max(2, math.ceil(left / 128))
        while left % r or left // r > 128 or r % 2:
            r += 1
            assert r <= left, f"cannot tile tail of {left} rows"
        plan.append((base, left // r, r))
    return plan


def _chunks(r):
    """Split R slots into even-sized chunks of at most 8 slots."""
    out = []
    j = 0
    while r - j > 8:
        out.append((j, 8))
        j += 8
    left = r - j
    if left > 0:
        if left % 2 == 0:
            out.append((j, left))
        else:
            raise AssertionError("odd chunk")
    return out




def _prune_redundant_waits(nc):
    """Drop sync waits provably implied (transitively) by other waits.

    This walrus encodes at most ONE sync wait per engine instruction. Tile's
    per-proc wait emission is not transitively minimal: e.g. a Matmult waits
    both on PE-self (psum bank WAW vs older matmuls) and on the DVE add that
    *read* those matmuls' output -- the DVE wait implies the PE one. We compute
    happens-before vector clocks over the emitted sync graph and delete waits
    that are covered by (a) the same-engine predecessor's knowledge or (b)
    another wait on the same instruction.
    """
    f = nc.m.functions[0]
    insts = []
    for bb in f.blocks:
        for inst in bb.instructions:
            insts.append(inst)

    # per-proc streams: engine streams in encounter order
    streams = {}
    pos = {}  # inst name -> (proc, idx)
    for inst in insts:
        eng = str(inst.engine)
        streams.setdefault(eng, []).append(inst)
        pos[inst.name] = (eng, len(streams[eng]) - 1)

    # semaphore update timeline: sem id -> list of (cum_value, inst_name)
    sem_updates = {}
    for inst in insts:
        si = inst.sync_info
        if si is None:
            continue
        for upd in si.on_update:
            if upd.sync_type != "semaphore" or upd.update_mode not in (
                "sem-inc",
                "sem-add-imm",
            ):
                continue
            lst = sem_updates.setdefault(upd.ant_name, [])
            prev = lst[-1][0] if lst else 0
            lst.append((prev + upd.update_value, inst.name))

    def satisfier(w):
        """instruction whose update satisfies wait w, or None."""
        if w.sync_type != "semaphore" or w.wait_mode != "sem-ge-imm":
            return None
        lst = sem_updates.get(w.ant_name)
        if not lst:
            return None
        for cum, nm in lst:
            if cum >= w.wait_value:
                return nm
        return None

    # vector clocks: map proc -> highest known retired index
    vc = {nm: {} for nm in pos}

    def join(dst, src):
        changed = False
        for k, v in src.items():
            if dst.get(k, -1) < v:
                dst[k] = v
                changed = True
        return changed

    for _ in range(16):
        changed = False
        for eng, stream in streams.items():
            run = {}
            for i, inst in enumerate(stream):
                nm = inst.name
                si = inst.sync_info
                if si is not None:
                    for w in si.on_wait:
                        s = satisfier(w)
                        if s is None:
                            continue
                        sp, sidx = pos[s]
                        join(run, vc[s])
                        if run.get(sp, -1) < sidx:
                            run[sp] = sidx
                if join(vc[nm], run):
                    changed = True
                join(run, {eng: i})
        if not changed:
            break

    # prune
    for eng, stream in streams.items():
        for i, inst in enumerate(stream):
            si = inst.sync_info
            if si is None or len(si.on_wait) <= 1:
                continue
            known = {}
            if i > 0:
                join(known, vc[stream[i - 1].name])
                join(known, {eng: i - 1})
            waits = list(si.on_wait)
            sats = [satisfier(w) for w in waits]
            keep = [True] * len(waits)
            # greedily try to drop waits that are covered
            for trial in range(len(waits)):
                dropped_any = False
                for j in range(len(waits)):
                    if not keep[j] or sats[j] is None:
                        continue
                    cover = dict(known)
                    for k in range(len(waits)):
                        if k == j or not keep[k] or sats[k] is None:
                            continue
                        join(cover, vc[sats[k]])
                        skp, skidx = pos[sats[k]]
                        if cover.get(skp, -1) < skidx:
                            cover[skp] = skidx
                    sp, sidx = pos[sats[j]]
                    if cover.get(sp, -1) >= sidx:
                        keep[j] = False
                        dropped_any = True
                if not dropped_any:
                    break
            new_waits = [w for w, k in zip(waits, keep) if k]
            if len(new_waits) != len(waits):
                inst.sync_info = mybir.SyncInfo(
                    on_wait=new_waits, on_update=list(si.on_update)
                )


def build_program(n_per=N_PER, num_devices=N_CORES, passes=1, ablate=()):
    """Build the Bass/Tile program for one core's shard of n_per rows."""
    plan = _plan(n_per)
    n_slots = sum(r for _, _, r in plan)
    max_pairs = max(r // 2 for _, _, r in plan)

    nc = bass.Bass(
        "TRN2",
        target_bir_lowering=False,
        debug=False,
        num_devices=num_devices,
    )

    x_d = nc.dram_tensor("x", [n_per, C], F32, kind="ExternalInput").ap()
    # per-supertile one-hot lhsT blocks: [n_st, 32, max_pairs*128]
    oht_d = nc.dram_tensor(
        "oht", [len(plan), 32, max_pairs * 128], BF16, kind="ExternalInput"
    ).ap()
    # host-baked one-hot target mask, fp8 (1.0 at column t_i, else 0);
    # fp8 halves the mask DMA stream vs bf16 (measured equal on the DVE side).
    mask_d = nc.dram_tensor("maskd", [n_per, C], MASK_DT, kind="ExternalInput").ap()
    w32_d = nc.dram_tensor("w32", [32, 2 * C], BF16, kind="ExternalInput").ap()
    out_d = nc.dram_tensor("out", [128, 2], F32, kind="ExternalOutput").ap()

    with ExitStack() as ctx:
        tc = ctx.enter_context(tile.TileContext(nc))

        singles = ctx.enter_context(tc.tile_pool(name="singles", bufs=1))
        xp = ctx.enter_context(tc.tile_pool(name="xp", bufs=6))
        pp = ctx.enter_context(tc.tile_pool(name="pp", bufs=2, space="PSUM"))
        sp = ctx.enter_context(tc.tile_pool(name="sp", bufs=4))
        ep = ctx.enter_context(tc.tile_pool(name="ep", bufs=3))
        mp = ctx.enter_context(tc.tile_pool(name="mp", bufs=4))
        absp = ctx.enter_context(tc.tile_pool(name="absp", bufs=8))
        vp = ctx.enter_context(tc.tile_pool(name="vp", bufs=3))

        w32_sb = singles.tile([32, 2 * C], BF16)
        nc.sync.dma_start(out=w32_sb[:], in_=w32_d[:, :])

        s_all = singles.tile([128, n_slots], F32)
        nc.vector.memset(s_all[:], 1.0)  # ln(1)=0 for unused cells
        vcols = singles.tile([128, len(plan)], F32)
        nc.vector.memset(vcols[:], 0.0)
        out_sb = singles.tile([128, 2], F32)

        # all one-hot lhsT blocks resident for the whole kernel: [32, n_st*mp*128]
        oht_all = singles.tile([32, len(plan) * max_pairs * 128], BF16)
        nc.sync.dma_start(
            out=oht_all[:].rearrange("k (s n) -> k s n", n=max_pairs * 128),
            in_=oht_d[:, :, :].rearrange("s k n -> k s n"),
        )

        for _pass in range(passes):
            col0 = 0
            for st, (base, P, R) in enumerate(plan):
                x_st = x_d[base : base + P * R, :].rearrange("(p r) c -> p r c", r=R)
                oht_sb = oht_all[:, st * max_pairs * 128 : (st + 1) * max_pairs * 128]

                scores = sp.tile([128, R * C], BF16)
                e_t = ep.tile([128, R * C], BF16)
                mask = mp.tile([128, R * C], MASK_DT)
                if "maskdma" not in ablate:
                    nc.sync.dma_start(
                        out=mask[:P].rearrange("p (r c) -> p r c", c=C),
                        in_=mask_d[base : base + P * R, :].rearrange(
                            "(p r) c -> p r c", r=R
                        ),
                    )

                for j0, cw in _chunks(R):
                    nf = cw * C
                    npair_c = cw // 2
                    x_t = xp.tile([128, nf], F32, tag="xt")
                    nc.sync.dma_start(
                        out=x_t[:P].rearrange("p (r c) -> p r c", c=C),
                        in_=x_st[:, j0 : j0 + cw, :],
                    )

                    psum_t = pp.tile([128, npair_c * 512], F32, tag="ps")
                    for pl, pr in enumerate(range(j0 // 2, (j0 + cw) // 2)):
                      if "mm" in ablate:
                        continue
                      else:
                        nc.tensor.matmul(
                            out=psum_t[:P, pl * 512 : pl * 512 + 2 * C],
                            lhsT=oht_sb[:, pr * 128 : pr * 128 + P],
                            rhs=w32_sb[:, :],
                            start=True,
                            stop=True,
                        )

                    # DVE wait-absorber: observe the x DMA on a [1,1] copy so the
                    # add itself only waits on the PE matmul (1-wait limit).
                    ascr = absp.tile([1, 1], F32, tag="ascr")
                    abs_i = nc.vector.tensor_copy(ascr[:, :], x_t[0:1, 0:1])
                    sc_c = scores[:P, j0 * C : (j0 + cw) * C]
                    if "add" in ablate:
                        continue
                    add_i = nc.vector.tensor_tensor(
                        out=sc_c.rearrange("p (r c) -> p r c", c=2 * C),
                        in0=x_t[:P].rearrange("p (r c) -> p r c", c=2 * C),
                        in1=psum_t[:P].rearrange("p (r c) -> p r c", c=512)[:, :, : 2 * C],
                        op=mybir.AluOpType.add,
                    )
                    add_dep_helper(add_i.ins, abs_i.ins, False, "add after x-absorber")
                    if "exp" not in ablate:
                        nc.scalar.activation(
                            e_t[:P, j0 * C : (j0 + cw) * C],
                            sc_c,
                            mybir.ActivationFunctionType.Exp,
                        )

                # whole-supertile ops
                if "reduce" not in ablate:
                    nc.vector.tensor_reduce(
                        out=s_all[:P, col0 : col0 + R],
                        in_=e_t[:P].rearrange("p (r c) -> p r c", c=C),
                        axis=mybir.AxisListType.X,
                        op=mybir.AluOpType.add,
                    )

                if "v" not in ablate:
                    # V extraction: masked = scores * mask on DVE (tensor_tensor_reduce
                    # is broken in this walrus), then ACT Copy-with-accum reduces it
                    # into this supertile's vcols column.
                    mscr = absp.tile([1, 1], F32, tag="mscr")
                    mabs_i = nc.vector.tensor_copy(mscr[:, :], mask[0:1, 0:1])
                    vscr = vp.tile([128, R * C], BF16)
                    vmul_i = nc.vector.tensor_tensor(
                        out=vscr[:P],
                        in0=scores[:P],
                        in1=mask[:P],
                        op=mybir.AluOpType.mult,
                    )
                    add_dep_helper(vmul_i.ins, mabs_i.ins, False, "vmul after mask-absorber")
                    nc.scalar.activation(
                        vscr[:P],
                        vscr[:P],
                        mybir.ActivationFunctionType.Copy,
                        accum_out=vcols[:P, st : st + 1],
                    )
                col0 += R

        ln_scr = singles.tile([128, n_slots], F32)
        nc.scalar.activation(
            ln_scr[:],
            s_all[:],
            mybir.ActivationFunctionType.Ln,
        )
        nc.vector.tensor_reduce(
            out=out_sb[:, 0:1],
            in_=ln_scr[:],
            axis=mybir.AxisListType.X,
            op=mybir.AluOpType.add,
        )
        nc.vector.tensor_reduce(
            out=out_sb[:, 1:2],
            in_=vcols[:],
            axis=mybir.AxisListType.X,
            op=mybir.AluOpType.add,
        )
        nc.sync.dma_start(out=out_d[:, :], in_=out_sb[:])

    _prune_redundant_waits(nc)

    # this walrus caps EVERY engine instruction at one sync wait. Verify.
    violations = []
    f = nc.m.functions[0]
    for bb in f.blocks:
        for inst in bb.instructions:
            si = inst.sync_info
            if si is None:
                continue
            nm = type(inst).__name__
            if nm in (
                "InstDrain",
                "InstEventSemaphore",
                "InstUnconditionalBranch",
                "InstRegisterMove",
                "InstCall",
                "InstNoOp",
            ):
                continue
            if len(si.on_wait) > 1:
                violations.append(
                    (
                        inst.name,
                        nm,
                        str(inst.engine),
                        [(w.ant_name, w.wait_value) for w in si.on_wait],
                    )
                )
    nc._wait_violations = violations

    return nc, plan, n_slots, max_pairs


def prep_inmaps(inputs, targets, domains, dcc_weights, n_cores, n_per):
    """Host-side index/table prep. O(N) integer work + tiny tables only."""
    plan = _plan(n_per)
    n_slots = sum(r for _, _, r in plan)
    max_pairs = max(r // 2 for _, _, r in plan)

    inputs = np.ascontiguousarray(np.asarray(inputs, dtype=np.float32))
    targets = np.asarray(targets).astype(np.int64).reshape(-1)
    domains = np.asarray(domains).astype(np.int64).reshape(-1)
    dcc = np.asarray(dcc_weights, dtype=np.float32)

    logw = np.full_like(dcc, -np.inf)
    np.log(dcc, out=logw, where=dcc > 0)
    w_hi = logw.astype(BF16_NP)
    w_lo = (logw - w_hi.astype(np.float32)).astype(BF16_NP)
    w32 = np.zeros((32, 2 * C), dtype=BF16_NP)
    w32[0:8, 0:C] = w_hi
    w32[8:16, 0:C] = w_lo
    w32[16:24, C : 2 * C] = w_hi
    w32[24:32, C : 2 * C] = w_lo

    ar8 = np.arange(D)

    in_maps = []
    for c in range(n_cores):
        sl = slice(c * n_per, (c + 1) * n_per)
        t_c = targets[sl]
        d_c = domains[sl]

        oht = np.zeros((len(plan), 32, max_pairs * 128), dtype=BF16_NP)
        maskd = np.zeros((n_per, C), dtype=MASK_DT_NP)
        valid = (t_c >= 0) & (t_c < C)
        maskd[np.nonzero(valid)[0], t_c[valid]] = 1.0
        for st, (base, P, R) in enumerate(plan):
            d_st = d_c[base : base + P * R].reshape(P, R)
            # one-hot lhsT blocks per slot pair
            oha = (d_st[:, 0::2, None] == ar8).astype(BF16_NP)  # [P, pairs, 8]
            ohb = (d_st[:, 1::2, None] == ar8).astype(BF16_NP)
            npair = R // 2
            blk = oht[st].reshape(32, max_pairs, 128)
            blk[0:8, :npair, :P] = np.transpose(oha, (2, 1, 0))
            blk[8:16, :npair, :P] = np.transpose(oha, (2, 1, 0))
            blk[16:24, :npair, :P] = np.transpose(ohb, (2, 1, 0))
            blk[24:32, :npair, :P] = np.transpose(ohb, (2, 1, 0))

        in_maps.append(
            {
                "x": inputs[sl],
                "oht": oht,
                "maskd": maskd,
                "w32": w32,
            }
        )
    return in_maps


def combine(results, n_total):
    """Combine per-core [128, 2] partials into the scalar loss."""
    total = 0.0
    for r in results:
        o = np.asarray(r["out"], dtype=np.float64)
        total += float(o[:, 0].sum() - o[:, 1].sum())
    return np.float32(total / n_total)


_PROGRAM_CACHE = {}


def _get_program(n_per, n_cores):
    key = (n_per, n_cores)
    if key not in _PROGRAM_CACHE:
        _PROGRAM_CACHE[key] = build_program(n_per, n_cores)
    return _PROGRAM_CACHE[key]


LAST_RESULT = None  # BassKernelResults of the most recent run (for profiling)


def run(inputs, targets, domains, dcc_weights, trace=False, tmpdir=None):
    global LAST_RESULT
    n = inputs.shape[0]
    assert n % N_CORES == 0
    n_per = n // N_CORES
    nc, _, _, _ = _get_program(n_per, N_CORES)
    in_maps = prep_inmaps(inputs, targets, domains, dcc_weights, N_CORES, n_per)
    res = run_bass_kernel_spmd(
        nc, in_maps, core_ids=list(range(N_CORES)), trace=trace, tmpdir=tmpdir
    )
    LAST_RESULT = res
    return combine(res.results, n)


def kernel(inputs, targets, domains, dcc_weights):
    targets = np.asarray(targets).reshape(-1)
    if np.any((targets < 0) | (targets >= C)):
        # IGNORE/out-of-range targets: exact but slow host fallback
        # (never hit for the spec'd input distribution).
        x = np.asarray(inputs, dtype=np.float64)
        dcc = np.asarray(dcc_weights, dtype=np.float64)
        logw = np.where(dcc > 0, np.log(np.maximum(dcc, 1e-300)), -np.inf)
        scores = logw[np.asarray(domains).reshape(-1)] + x
        m = scores.max(axis=1)
        lse = m + np.log(np.exp(scores - m[:, None]).sum(axis=1))
        tgt = np.clip(targets, 0, C - 1)
        ts = scores[np.arange(x.shape[0]), tgt]
        valid = targets != IGNORE
        return np.float32(np.where(valid, lse - ts, 0.0).sum() / x.shape[0])
    return run(inputs, targets, domains, dcc_weights, trace=False)



# revision 4
# speedup vs baseline: 3.1707x; 3.1707x over previous
"""Trainium2 Bass kernel for DomainCalibratedLoss (v2).

loss = [ sum_i ln(sum_j w[d_i,j] e^{x_ij}) - sum_i (x[i,t_i] + ln w[d_i,t_i]) ] / N

Device computes the heavy term sum_i ln S_i with S_i = sum_j w[d_i,j] e^{x_ij}.
The target-score sum is an O(N) exact f64 gather on the host, and x is
delivered to the device as bf16 (validated: total rel err ~1e-4 vs the f64
reference, gate is 2e-2).

Layout/strategy (per core, data-parallel over 8 cores):
  * host sorts rows by domain and pads each domain to a multiple of T=512,
    giving domain-pure chunks of 512 rows; 1024 chunks globally, 128/core
    (pad rows have x=0 and are corrected exactly on the host).
  * x ships TRANSPOSED: xT[chunk, group, class, row] bf16, so each chunk is
    two [100, 512] class-major tiles -- PE contracts over classes.
  * exp is split across two engines (both bias-free):
      - ACT chunks: true exp, one in-place [100, 1024] Activation.
      - DVE chunks: int16 Schraudolph exp~ = bitcast_bf16(int16(x*(2^7/ln2)+B)),
        one fused in-place tensor_scalar (mult+add, int16 out) at 2x rate.
  * PE fuses the weighted class-reduce: S row = w_d^T @ e via two K=100
    matmuls per chunk (lhsT = per-chunk host-filled w column, bf16).
    Chunk s of a 12-chunk round lands in psum bank s//3 at partition 32*(s%3)
    (matmul out base partitions are restricted to {0,32,64}).
  * one ACT Ln per round reads S straight from PSUM [65, 4*512] and
    accum_out's each partition-lane; host sums lanes {0,32,64} per round.
    Garbage lanes keep their garbage -- never read.

Walrus caps every engine instruction at ONE sync wait; deps are arranged so
Tile's waits are transitively implied (ACT-half chunks open each round so the
psum-WAR wait on the previous Ln is implied), then _prune_redundant_waits
deletes the redundant ones.
"""

import numpy as np
from contextlib import ExitStack

import concourse.bass as bass
import concourse.tile as tile
from concourse import mybir
from concourse.tile_rust import add_dep_helper
from concourse.bass_utils import run_bass_kernel_spmd

F32 = mybir.dt.float32
BF16 = mybir.dt.bfloat16
I16 = mybir.dt.int16
BF16_NP = mybir.dt.np(BF16)

N_TOTAL = 500000
N_CORES = 8
C = 200
CG = 100  # classes per contraction group
D = 8
IGNORE = 255
T = 512  # rows per chunk
CHUNKS = 128  # chunks per core
N_PER = CHUNKS * T  # padded rows per core (65536)
ROUND = 12  # chunks per psum round (4 banks x 3 quadrant lanes)
ACT_PER_ROUND = 3  # leading chunks of each round computed on ACT (rest DVE)

LOG2E = float(np.log2(np.e))
A16 = float(np.float32((2 ** 7) * LOG2E))
B16_C = 7.5
B16 = float(np.float32(127 * 2 ** 7 - B16_C))


def _rounds():
    """[(round_chunks, [(col, part_hi, bank_lo, bank_hi, lanes)])] structure.

    Each round of up to 12 chunks maps chunk s -> bank s//3, lane 32*(s%3).
    Ln windows per round: one over the full banks (all 3 lanes written) and,
    for a ragged tail, one over the final partial bank.
    """
    rounds = []
    col = 0
    left = CHUNKS
    while left > 0:
        n = min(ROUND, left)
        fb, rem = n // 3, n % 3
        wins = []
        if fb:
            wins.append((col, 65, 0, fb, (0, 32, 64)))
            col += 1
        if rem:
            wins.append((col, 32 * (rem - 1) + 1, fb, fb + 1, tuple(32 * q for q in range(rem))))
            col += 1
        rounds.append((n, wins))
        left -= n
    return rounds, col


ROUNDS, N_COLS = _rounds()


def _prune_redundant_waits(nc):
    """Drop sync waits provably implied (transitively) by other waits.

    Walrus encodes at most ONE sync wait per engine instruction. Tile's
    per-proc wait emission is not transitively minimal. We compute
    happens-before vector clocks over the emitted sync graph and delete waits
    covered by (a) the same-engine predecessor's knowledge or (b) another
    wait on the same instruction.
    """
    f = nc.m.functions[0]
    insts = []
    for bb in f.blocks:
        for inst in bb.instructions:
            insts.append(inst)

    streams = {}
    pos = {}
    for inst in insts:
        eng = str(inst.engine)
        streams.setdefault(eng, []).append(inst)
        pos[inst.name] = (eng, len(streams[eng]) - 1)

    sem_updates = {}
    for inst in insts:
        si = inst.sync_info
        if si is None:
            continue
        for upd in si.on_update:
            if upd.sync_type != "semaphore" or upd.update_mode not in (
                "sem-inc",
                "sem-add-imm",
            ):
                continue
            lst = sem_updates.setdefault(upd.ant_name, [])
            prev = lst[-1][0] if lst else 0
            lst.append((prev + upd.update_value, inst.name))

    def satisfier(w):
        if w.sync_type != "semaphore" or w.wait_mode != "sem-ge-imm":
            return None
        lst = sem_updates.get(w.ant_name)
        if not lst:
            return None
        for cum, nm in lst:
            if cum >= w.wait_value:
                return nm
        return None

    vc = {nm: {} for nm in pos}

    def join(dst, src):
        changed = False
        for k, v in src.items():
            if dst.get(k, -1) < v:
                dst[k] = v
                changed = True
        return changed

    for _ in range(16):
        changed = False
        for eng, stream in streams.items():
            run = {}
            for i, inst in enumerate(stream):
                nm = inst.name
                si = inst.sync_info
                if si is not None:
                    for w in si.on_wait:
                        s = satisfier(w)
                        if s is None:
                            continue
                        sp, sidx = pos[s]
                        join(run, vc[s])
                        if run.get(sp, -1) < sidx:
                            run[sp] = sidx
                if join(vc[nm], run):
                    changed = True
                join(run, {eng: i})
        if not changed:
            break

    for eng, stream in streams.items():
        for i, inst in enumerate(stream):
            si = inst.sync_info
            if si is None or len(si.on_wait) <= 1:
                continue
            known = {}
            if i > 0:
                join(known, vc[stream[i - 1].name])
                join(known, {eng: i - 1})
            waits = list(si.on_wait)
            sats = [satisfier(w) for w in waits]
            keep = [True] * len(waits)
            for _trial in range(len(waits)):
                dropped_any = False
                for j in range(len(waits)):
                    if not keep[j] or sats[j] is None:
                        continue
                    cover = dict(known)
                    for k in range(len(waits)):
                        if k == j or not keep[k] or sats[k] is None:
                            continue
                        join(cover, vc[sats[k]])
                        skp, skidx = pos[sats[k]]
                        if cover.get(skp, -1) < skidx:
                            cover[skp] = skidx
                    sp, sidx = pos[sats[j]]
                    if cover.get(sp, -1) >= sidx:
                        keep[j] = False
                        dropped_any = True
                if not dropped_any:
                    break
            new_waits = [w for w, k in zip(waits, keep) if k]
            if len(new_waits) != len(waits):
                inst.sync_info = mybir.SyncInfo(
                    on_wait=new_waits, on_update=list(si.on_update)
                )


def build_program(num_devices=N_CORES, passes=1, act_per_round=ACT_PER_ROUND,
                  ablate=()):
    nc = bass.Bass(
        "TRN2",
        target_bir_lowering=False,
        debug=False,
        num_devices=num_devices,
    )

    xt_d = nc.dram_tensor("xt", [CHUNKS, CG, 2 * T], BF16, kind="ExternalInput").ap()
    w_d = nc.dram_tensor("wall", [CG, 2 * CHUNKS], BF16, kind="ExternalInput").ap()
    out_d = nc.dram_tensor("out", [65, 16], F32, kind="ExternalOutput").ap()

    with ExitStack() as ctx:
        tc = ctx.enter_context(tile.TileContext(nc))

        singles = ctx.enter_context(tc.tile_pool(name="singles", bufs=1))
        xp = ctx.enter_context(tc.tile_pool(name="xp", bufs=6))
        pp = ctx.enter_context(tc.tile_pool(name="pp", bufs=2, space="PSUM"))
        absp = ctx.enter_context(tc.tile_pool(name="absp", bufs=2))

        w_all = singles.tile([CG, 2 * CHUNKS], BF16)
        nc.sync.dma_start(out=w_all[:], in_=w_d[:, :])
        ln_scr = singles.tile([65, 4 * T], F32)
        acc = singles.tile([65, 16], F32)

        ln_hist = []  # Ln per round across passes (psum pool has 2 bufs)
        for _pass in range(passes):
            c = 0
            for r, (n_chunks, wins) in enumerate(ROUNDS):
                ps = pp.tile([128, 4 * T], F32, tag="ps")
                dve_abs = None
                for s in range(n_chunks):
                    b, q = s // 3, s % 3
                    xg = xp.tile([CG, 2 * T], BF16, tag="xg")
                    if "dma" not in ablate:
                        nc.sync.dma_start(out=xg[:], in_=xt_d[c, :, :])
                    is_act = s < act_per_round
                    if is_act:
                        if "exp" not in ablate:
                            nc.scalar.activation(
                                xg[:], xg[:], mybir.ActivationFunctionType.Exp
                            )
                    else:
                        if "ts" not in ablate:
                            ts_i = nc.vector.tensor_scalar(
                                out=xg[:].bitcast(I16),
                                in0=xg[:],
                                scalar1=A16,
                                scalar2=B16,
                                op0=mybir.AluOpType.mult,
                                op1=mybir.AluOpType.add,
                            )
                            # chain the first DVE op of the round behind the
                            # Ln that last read this psum tile, so the matmul's
                            # psum-WAR wait is implied (1-wait cap).
                            if dve_abs is None and len(ln_hist) >= 2:
                                add_dep_helper(
                                    ts_i.ins, ln_hist[-2].ins, False,
                                    "round ts after psum Ln",
                                )
                                dve_abs = ts_i
                    if "mm" not in ablate:
                        for g in range(2):
                            nc.tensor.matmul(
                                out=ps[32 * q : 32 * q + 1, T * b : T * (b + 1)],
                                lhsT=w_all[:, 2 * c + g : 2 * c + g + 1],
                                rhs=xg[:, T * g : T * (g + 1)],
                                start=(g == 0),
                                stop=(g == 1),
                                skip_group_check=True,
                            )
                    c += 1
                if "ln" not in ablate:
                    last = None
                    for col, phi, blo, bhi, _lanes in wins:
                        last = nc.scalar.activation(
                            ln_scr[0:phi, T * blo : T * bhi],
                            ps[0:phi, T * blo : T * bhi],
                            mybir.ActivationFunctionType.Ln,
                            accum_out=acc[0:phi, col : col + 1],
                        )
                    ln_hist.append(last)

        nc.sync.dma_start(out=out_d[:, :], in_=acc[:])

    _prune_redundant_waits(nc)

    violations = []
    f = nc.m.functions[0]
    n_table_loads = 0
    for bb in f.blocks:
        for inst in bb.instructions:
            if type(inst).__name__ == "InstLoadActFuncSet":
                n_table_loads += 1
            si = inst.sync_info
            if si is None:
                continue
            nm = type(inst).__name__
            if nm in (
                "InstDrain",
                "InstEventSemaphore",
                "InstUnconditionalBranch",
                "InstRegisterMove",
                "InstCall",
                "InstNoOp",
            ):
                continue
            if len(si.on_wait) > 1:
                violations.append(
                    (inst.name, nm, str(inst.engine),
                     [(w.ant_name, w.wait_value) for w in si.on_wait])
                )
    nc._wait_violations = violations
    nc._n_table_loads = n_table_loads
    return nc


def prep_inmaps(inputs, targets, domains, dcc_weights, act_per_round=ACT_PER_ROUND):
    """Host prep: domain sort + pad, bf16 transpose-chunk x, per-chunk w
    columns, exact f64 target-score sum, pad corrections."""
    x = np.ascontiguousarray(np.asarray(inputs, dtype=np.float32))
    t = np.asarray(targets).astype(np.int64).reshape(-1)
    d = np.asarray(domains).astype(np.int64).reshape(-1)
    w = np.asarray(dcc_weights, dtype=np.float32)
    n = x.shape[0]

    order = np.argsort(d, kind="stable")
    counts = np.bincount(d[order], minlength=D)
    total_chunks = N_CORES * CHUNKS

    gidx = np.full(total_chunks * T, -1, np.int64)
    chunk_dom = np.zeros(total_chunks, np.int64)
    pos = 0
    src = 0
    for dd in range(D):
        cnt = int(counts[dd])
        m = (cnt + T - 1) // T
        gidx[pos : pos + cnt] = order[src : src + cnt]
        chunk_dom[pos // T : pos // T + m] = dd
        pos += m * T
        src += cnt
    assert pos // T <= total_chunks

    w_bf = w.astype(BF16_NP)
    w_bf_f = w_bf.astype(np.float32)

    s_in_round = np.arange(CHUNKS) % ROUND
    is_act_chunk = s_in_round < act_per_round  # same pattern every core

    # pad correction (exact): pad rows have x=0
    e_pad_act = 1.0
    e_pad_dve = float(
        np.round(np.float32(0) * np.float32(A16) + np.float32(B16))
        .astype(np.int16).view(BF16_NP)
    )
    pad_corr = 0.0
    npads = np.bincount(
        np.nonzero(gidx < 0)[0] // T, minlength=total_chunks
    )
    for cidx in np.nonzero(npads)[0]:
        ep = e_pad_act if is_act_chunk[cidx % CHUNKS] else e_pad_dve
        s_pad = float((w_bf_f[chunk_dom[cidx]] * np.float32(ep)).sum(dtype=np.float32))
        pad_corr += float(npads[cidx]) * float(np.log(s_pad))

    # exact host-side target-score sum (f64, from the original f32 x)
    logw = np.log(w.astype(np.float64))
    tc_ = np.clip(t, 0, C - 1)
    valid = t != IGNORE
    t_sum = float(
        np.where(valid, x[np.arange(n), tc_].astype(np.float64) + logw[d, tc_], 0.0).sum()
    )

    in_maps = []
    for core in range(N_CORES):
        sl = slice(core * N_PER, (core + 1) * N_PER)
        gi = gidx[sl]
        xb = np.zeros((N_PER, C), BF16_NP)
        v = gi >= 0
        xb[v] = x[gi[v]].astype(BF16_NP)
        xt = np.ascontiguousarray(
            xb.reshape(CHUNKS, T, 2, CG).transpose(0, 3, 2, 1)
        ).reshape(CHUNKS, CG, 2 * T)
        doms = chunk_dom[core * CHUNKS : (core + 1) * CHUNKS]
        wall = np.empty((CG, 2 * CHUNKS), BF16_NP)
        wall[:, 0::2] = w_bf[doms, 0:CG].T
        wall[:, 1::2] = w_bf[doms, CG:C].T
        in_maps.append({"xt": xt, "wall": wall})

    host_terms = (pad_corr, t_sum)
    return in_maps, host_terms


def combine(results, host_terms, n_total):
    pad_corr, t_sum = host_terms
    total_ln = 0.0
    for r in results:
        o = np.asarray(r["out"], dtype=np.float64)
        for _n_chunks, wins in ROUNDS:
            for col, _phi, _blo, _bhi, lanes in wins:
                total_ln += o[list(lanes), col].sum()
    return np.float32((total_ln - pad_corr - t_sum) / n_total)


_PROGRAM_CACHE = {}


def _get_program():
    key = "v2"
    if key not in _PROGRAM_CACHE:
        _PROGRAM_CACHE[key] = build_program()
    return _PROGRAM_CACHE[key]


LAST_RESULT = None


def run(inputs, targets, domains, dcc_weights, trace=False, tmpdir=None):
    global LAST_RESULT
    n = inputs.shape[0]
    assert n == N_TOTAL
    nc = _get_program()
    in_maps, host_terms = prep_inmaps(inputs, targets, domains, dcc_weights)
    res = run_bass_kernel_spmd(
        nc, in_maps, core_ids=list(range(N_CORES)), trace=trace, tmpdir=tmpdir
    )
    LAST_RESULT = res
    return combine(res.results, host_terms, n)


def kernel(inputs, targets, domains, dcc_weights):
    targets = np.asarray(targets).reshape(-1)
    if np.any((targets < 0) | (targets >= C)) or inputs.shape[0] != N_TOTAL:
        # IGNORE/out-of-range targets or odd shapes: exact host fallback
        # (never hit for the spec'd input distribution).
        x = np.asarray(inputs, dtype=np.float64)
        dcc = np.asarray(dcc_weights, dtype=np.float64)
        logw = np.where(dcc > 0, np.log(np.maximum(dcc, 1e-300)), -np.inf)
        scores = logw[np.asarray(domains).reshape(-1)] + x
        m = scores.max(axis=1)
        lse = m + np.log(np.exp(scores - m[:, None]).sum(axis=1))
        tgt = np.clip(targets, 0, C - 1)
        ts = scores[np.arange(x.shape[0]), tgt]
        valid = targets != IGNORE
        return np.float32(np.where(valid, lse - ts, 0.0).sum() / x.shape[0])
    return run(inputs, targets, domains, dcc_weights, trace=False)


# revision 12
# speedup vs baseline: 3.5161x; 1.1089x over previous
"""Trainium2 Bass kernel for DomainCalibratedLoss (v2).

loss = [ sum_i ln(sum_j w[d_i,j] e^{x_ij}) - sum_i (x[i,t_i] + ln w[d_i,t_i]) ] / N

Device computes the heavy term sum_i ln S_i with S_i = sum_j w[d_i,j] e^{x_ij}.
The target-score sum is an O(N) exact f64 gather on the host, and x is
delivered to the device as bf16 (validated: total rel err ~1e-4 vs the f64
reference, gate is 2e-2).

Layout/strategy (data-parallel over 8 cores, DMA-bound):
  * host sorts rows by domain and pads each domain to a multiple of T=512,
    giving domain-pure 512-row chunks; 123 chunks/core (ceil(N/T)+D-1 <= 984
    covers any domain mix). Pad rows have x=0 and are corrected exactly on
    the host.
  * x ships TRANSPOSED and group-major: xt[group, class, 4*2T] bf16, so each
    x DMA reads one contiguous 8KB run per partition (4 chunks).
  * exp runs entirely on DVE as an int16 Schraudolph exponential:
    e~ = bitcast_bf16(int16(x*(2^7*log2 e) + B16)), one fused in-place
    tensor_scalar (mult+add) per chunk at 2x rate. B16 is tuned so the
    resulting bias on the final loss is ~1e-4. (True exp on ACT was tried
    and removed: interleaving Exp and Ln on ACT thrashes the 2.7us
    activation-table loads.)
  * PE fuses the weight-multiply and class-reduce: S row = w_d^T @ e via two
    K=100 bf16 matmuls per chunk (lhsT = per-chunk host-filled w column).
    Chunk s of a 12-chunk round lands in psum bank s//3 at partition
    32*(s%3) (matmul out base partitions are restricted to {0,32,64}).
  * one ACT Ln per round reads S straight from PSUM [65, 4*512] and
    accum_out's each partition lane; the host sums lanes {0,32,64} per
    round column. Garbage lanes keep their garbage -- never read.

Walrus caps every engine instruction at ONE sync wait. A tiny DVE "absorber"
copy at each round start observes the Ln whose psum buffer the round reuses,
making the matmuls' psum-WAR waits transitively implied; then
_prune_redundant_waits deletes the redundant waits (asserted zero left).
"""

import numpy as np
from contextlib import ExitStack

import concourse.bass as bass
import concourse.tile as tile
from concourse import mybir
from concourse.bass_utils import run_bass_kernel_spmd

F32 = mybir.dt.float32
BF16 = mybir.dt.bfloat16
I16 = mybir.dt.int16
BF16_NP = mybir.dt.np(BF16)

N_TOTAL = 500000
N_CORES = 8
C = 200
CG = 100  # classes per contraction group
D = 8
IGNORE = 255
T = 512  # rows per chunk
CHUNKS = 123  # chunks per core (ceil(N/T) + D - 1 <= 8*123 for any domain mix)
N_PER = CHUNKS * T  # padded rows per core (65536)
ROUND = 12  # chunks per psum round (4 banks x 3 quadrant lanes)
ACT_PER_ROUND = 0  # all chunks on DVE (ACT only runs Ln; avoids exp<->ln table thrash)
DMA_GROUP = 4  # chunks fetched per x DMA (group-major DRAM: 8KB/partition runs)
NGRP = (CHUNKS + DMA_GROUP - 1) // DMA_GROUP  # x DMA groups per core

LOG2E = float(np.log2(np.e))
A16 = float(np.float32((2 ** 7) * LOG2E))
B16_C = 7.5
B16 = float(np.float32(127 * 2 ** 7 - B16_C))


def _rounds():
    """[(round_chunks, [(col, part_hi, bank_lo, bank_hi, lanes)])] structure.

    Each round of up to 12 chunks maps chunk s -> bank s//3, lane 32*(s%3).
    Ln windows per round: one over the full banks (all 3 lanes written) and,
    for a ragged tail, one over the final partial bank.
    """
    rounds = []
    col = 0
    left = CHUNKS
    while left > 0:
        n = min(ROUND, left)
        fb, rem = n // 3, n % 3
        wins = []
        if fb:
            wins.append((col, 65, 0, fb, (0, 32, 64)))
            col += 1
        if rem:
            wins.append((col, 32 * (rem - 1) + 1, fb, fb + 1, tuple(32 * q for q in range(rem))))
            col += 1
        rounds.append((n, wins))
        left -= n
    return rounds, col


ROUNDS, N_COLS = _rounds()


def _prune_redundant_waits(nc):
    """Drop sync waits provably implied (transitively) by other waits.

    Walrus encodes at most ONE sync wait per engine instruction. Tile's
    per-proc wait emission is not transitively minimal. We compute
    happens-before vector clocks over the emitted sync graph and delete waits
    covered by (a) the same-engine predecessor's knowledge or (b) another
    wait on the same instruction.
    """
    f = nc.m.functions[0]
    insts = []
    for bb in f.blocks:
        for inst in bb.instructions:
            insts.append(inst)

    streams = {}
    pos = {}
    for inst in insts:
        eng = str(inst.engine)
        streams.setdefault(eng, []).append(inst)
        pos[inst.name] = (eng, len(streams[eng]) - 1)

    sem_updates = {}
    for inst in insts:
        si = inst.sync_info
        if si is None:
            continue
        for upd in si.on_update:
            if upd.sync_type != "semaphore" or upd.update_mode not in (
                "sem-inc",
                "sem-add-imm",
            ):
                continue
            lst = sem_updates.setdefault(upd.ant_name, [])
            prev = lst[-1][0] if lst else 0
            lst.append((prev + upd.update_value, inst.name))

    def satisfier(w):
        if w.sync_type != "semaphore" or w.wait_mode != "sem-ge-imm":
            return None
        lst = sem_updates.get(w.ant_name)
        if not lst:
            return None
        for cum, nm in lst:
            if cum >= w.wait_value:
                return nm
        return None

    vc = {nm: {} for nm in pos}

    def join(dst, src):
        changed = False
        for k, v in src.items():
            if dst.get(k, -1) < v:
                dst[k] = v
                changed = True
        return changed

    for _ in range(16):
        changed = False
        for eng, stream in streams.items():
            run = {}
            for i, inst in enumerate(stream):
                nm = inst.name
                si = inst.sync_info
                if si is not None:
                    for w in si.on_wait:
                        s = satisfier(w)
                        if s is None:
                            continue
                        sp, sidx = pos[s]
                        join(run, vc[s])
                        if run.get(sp, -1) < sidx:
                            run[sp] = sidx
                if join(vc[nm], run):
                    changed = True
                join(run, {eng: i})
        if not changed:
            break

    for eng, stream in streams.items():
        for i, inst in enumerate(stream):
            si = inst.sync_info
            if si is None or len(si.on_wait) <= 1:
                continue
            known = {}
            if i > 0:
                join(known, vc[stream[i - 1].name])
                join(known, {eng: i - 1})
            waits = list(si.on_wait)
            sats = [satisfier(w) for w in waits]
            keep = [True] * len(waits)
            for _trial in range(len(waits)):
                dropped_any = False
                for j in range(len(waits)):
                    if not keep[j] or sats[j] is None:
                        continue
                    cover = dict(known)
                    for k in range(len(waits)):
                        if k == j or not keep[k] or sats[k] is None:
                            continue
                        join(cover, vc[sats[k]])
                        skp, skidx = pos[sats[k]]
                        if cover.get(skp, -1) < skidx:
                            cover[skp] = skidx
                    sp, sidx = pos[sats[j]]
                    if cover.get(sp, -1) >= sidx:
                        keep[j] = False
                        dropped_any = True
                if not dropped_any:
                    break
            new_waits = [w for w, k in zip(waits, keep) if k]
            if len(new_waits) != len(waits):
                inst.sync_info = mybir.SyncInfo(
                    on_wait=new_waits, on_update=list(si.on_update)
                )


def build_program(num_devices=N_CORES, passes=1, act_per_round=ACT_PER_ROUND,
                  dma_group=DMA_GROUP, ablate=()):
    nc = bass.Bass(
        "TRN2",
        target_bir_lowering=False,
        debug=False,
        num_devices=num_devices,
    )

    xt_d = nc.dram_tensor("xt", [NGRP, CG, DMA_GROUP * 2 * T], BF16, kind="ExternalInput").ap()
    w_d = nc.dram_tensor("wall", [CG, 2 * CHUNKS], BF16, kind="ExternalInput").ap()
    out_d = nc.dram_tensor("out", [65, 16], F32, kind="ExternalOutput").ap()

    with ExitStack() as ctx:
        tc = ctx.enter_context(tile.TileContext(nc))

        singles = ctx.enter_context(tc.tile_pool(name="singles", bufs=1))
        xp = ctx.enter_context(tc.tile_pool(name="xp", bufs=6))
        pp = ctx.enter_context(tc.tile_pool(name="pp", bufs=2, space="PSUM"))
        absp = ctx.enter_context(tc.tile_pool(name="absp", bufs=4))

        w_all = singles.tile([CG, 2 * CHUNKS], BF16)
        nc.sync.dma_start(out=w_all[:], in_=w_d[:, :])
        ln_scr = singles.tile([65, 4 * T], F32)
        acc = singles.tile([65, 16], F32)
        if "ln" in ablate:
            nc.vector.memset(acc[:], 0.0)
            nc.scalar.activation(ln_scr[:], ln_scr[:].bitcast(F32),
                                 mybir.ActivationFunctionType.Copy) if False else None
            nc.vector.memset(ln_scr[:], 0.0)

        ln_cols = []  # first Ln accum column per emitted round (across passes)
        for _pass in range(passes):
            c = 0
            for r, (n_chunks, wins) in enumerate(ROUNDS):
                ps = pp.tile([128, 4 * T], F32, tag="ps")
                if len(ln_cols) >= 2 and "ln" not in ablate:
                    # DVE absorber: observe the Ln whose psum buffer this
                    # round reuses, so the first matmul's psum-WAR wait is
                    # transitively implied through the DVE exp (1-wait cap).
                    ascr = absp.tile([1, 1], F32, tag="ascr")
                    nc.vector.tensor_copy(
                        ascr[:, :], acc[0:1, ln_cols[-2] : ln_cols[-2] + 1]
                    )
                for s in range(n_chunks):
                    b, q = s // 3, s % 3
                    if c % DMA_GROUP == 0:
                        ng = min(DMA_GROUP, CHUNKS - c)
                        xgrp = xp.tile([CG, DMA_GROUP * 2 * T], BF16, tag="xg")
                        if "dma" not in ablate:
                            nc.sync.dma_start(
                                out=xgrp[:, : ng * 2 * T],
                                in_=xt_d[c // DMA_GROUP, :, : ng * 2 * T],
                            )
                        else:
                            nc.sync.dma_start(
                                out=xgrp[0:1, 0:16],
                                in_=xt_d[c // DMA_GROUP, 0:1, 0:16],
                            )
                    xg = xgrp[:, (c % DMA_GROUP) * 2 * T : (c % DMA_GROUP + 1) * 2 * T]
                    is_act = s < act_per_round
                    if is_act:
                        if "exp" not in ablate:
                            nc.scalar.activation(
                                xg, xg, mybir.ActivationFunctionType.Exp
                            )
                    else:
                        if "ts" not in ablate:
                            nc.vector.tensor_scalar(
                                out=xg.bitcast(I16),
                                in0=xg,
                                scalar1=A16,
                                scalar2=B16,
                                op0=mybir.AluOpType.mult,
                                op1=mybir.AluOpType.add,
                            )
                    if "mm" not in ablate:
                        for g in range(2):
                            nc.tensor.matmul(
                                out=ps[32 * q : 32 * q + 1, T * b : T * (b + 1)],
                                lhsT=w_all[:, 2 * c + g : 2 * c + g + 1],
                                rhs=xg[:, T * g : T * (g + 1)],
                                start=(g == 0),
                                stop=(g == 1),
                                skip_group_check=True,
                            )
                    c += 1
                if "ln" not in ablate:
                    for col, phi, blo, bhi, _lanes in wins:
                        nc.scalar.activation(
                            ln_scr[0:phi, T * blo : T * bhi],
                            ps[0:phi, T * blo : T * bhi],
                            mybir.ActivationFunctionType.Ln,
                            accum_out=acc[0:phi, col : col + 1],
                        )
                    ln_cols.append(wins[-1][0])

        nc.sync.dma_start(out=out_d[:, :], in_=acc[:])

    _prune_redundant_waits(nc)

    violations = []
    f = nc.m.functions[0]
    n_table_loads = 0
    for bb in f.blocks:
        for inst in bb.instructions:
            if type(inst).__name__ == "InstLoadActFuncSet":
                n_table_loads += 1
            si = inst.sync_info
            if si is None:
                continue
            nm = type(inst).__name__
            if nm in (
                "InstDrain",
                "InstEventSemaphore",
                "InstUnconditionalBranch",
                "InstRegisterMove",
                "InstCall",
                "InstNoOp",
            ):
                continue
            if len(si.on_wait) > 1:
                violations.append(
                    (inst.name, nm, str(inst.engine),
                     [(w.ant_name, w.wait_value) for w in si.on_wait])
                )
    nc._wait_violations = violations
    nc._n_table_loads = n_table_loads
    return nc


def prep_inmaps(inputs, targets, domains, dcc_weights, act_per_round=ACT_PER_ROUND):
    """Host prep: domain sort + pad, bf16 transpose-chunk x, per-chunk w
    columns, exact f64 target-score sum, pad corrections."""
    x = np.ascontiguousarray(np.asarray(inputs, dtype=np.float32))
    t = np.asarray(targets).astype(np.int64).reshape(-1)
    d = np.asarray(domains).astype(np.int64).reshape(-1)
    w = np.asarray(dcc_weights, dtype=np.float32)
    n = x.shape[0]

    order = np.argsort(d, kind="stable")
    counts = np.bincount(d[order], minlength=D)
    total_chunks = N_CORES * CHUNKS

    gidx = np.full(total_chunks * T, -1, np.int64)
    chunk_dom = np.zeros(total_chunks, np.int64)
    pos = 0
    src = 0
    for dd in range(D):
        cnt = int(counts[dd])
        m = (cnt + T - 1) // T
        gidx[pos : pos + cnt] = order[src : src + cnt]
        chunk_dom[pos // T : pos // T + m] = dd
        pos += m * T
        src += cnt
    assert pos // T <= total_chunks

    w_bf = w.astype(BF16_NP)
    w_bf_f = w_bf.astype(np.float32)

    s_in_round = np.arange(CHUNKS) % ROUND
    is_act_chunk = s_in_round < act_per_round  # same pattern every core

    # pad correction (exact): pad rows have x=0
    e_pad_act = 1.0
    e_pad_dve = float(
        np.round(np.float32(0) * np.float32(A16) + np.float32(B16))
        .astype(np.int16).view(BF16_NP)
    )
    pad_corr = 0.0
    npads = np.bincount(
        np.nonzero(gidx < 0)[0] // T, minlength=total_chunks
    )
    for cidx in np.nonzero(npads)[0]:
        ep = e_pad_act if is_act_chunk[cidx % CHUNKS] else e_pad_dve
        s_pad = float((w_bf_f[chunk_dom[cidx]] * np.float32(ep)).sum(dtype=np.float32))
        pad_corr += float(npads[cidx]) * float(np.log(s_pad))

    # exact host-side target-score sum (f64, from the original f32 x)
    logw = np.log(w.astype(np.float64))
    tc_ = np.clip(t, 0, C - 1)
    valid = t != IGNORE
    t_sum = float(
        np.where(valid, x[np.arange(n), tc_].astype(np.float64) + logw[d, tc_], 0.0).sum()
    )

    in_maps = []
    for core in range(N_CORES):
        sl = slice(core * N_PER, (core + 1) * N_PER)
        gi = gidx[sl]
        xb = np.zeros((N_PER, C), BF16_NP)
        v = gi >= 0
        xb[v] = x[gi[v]].astype(BF16_NP)
        xtc = xb.reshape(CHUNKS, T, 2, CG).transpose(0, 3, 2, 1).reshape(
            CHUNKS, CG, 2 * T
        )
        pad_slots = NGRP * DMA_GROUP - CHUNKS
        if pad_slots:
            xtc = np.concatenate(
                [xtc, np.zeros((pad_slots, CG, 2 * T), BF16_NP)], axis=0
            )
        xt = np.ascontiguousarray(
            xtc.reshape(NGRP, DMA_GROUP, CG, 2 * T).transpose(0, 2, 1, 3)
        ).reshape(NGRP, CG, DMA_GROUP * 2 * T)
        doms = chunk_dom[core * CHUNKS : (core + 1) * CHUNKS]
        wall = np.empty((CG, 2 * CHUNKS), BF16_NP)
        wall[:, 0::2] = w_bf[doms, 0:CG].T
        wall[:, 1::2] = w_bf[doms, CG:C].T
        in_maps.append({"xt": xt, "wall": wall})

    host_terms = (pad_corr, t_sum)
    return in_maps, host_terms


def combine(results, host_terms, n_total):
    pad_corr, t_sum = host_terms
    total_ln = 0.0
    for r in results:
        o = np.asarray(r["out"], dtype=np.float64)
        for _n_chunks, wins in ROUNDS:
            for col, _phi, _blo, _bhi, lanes in wins:
                total_ln += o[list(lanes), col].sum()
    return np.float32((total_ln - pad_corr - t_sum) / n_total)


_PROGRAM_CACHE = {}


def _get_program():
    key = "v2"
    if key not in _PROGRAM_CACHE:
        _PROGRAM_CACHE[key] = build_program()
    return _PROGRAM_CACHE[key]


LAST_RESULT = None


def run(inputs, targets, domains, dcc_weights, trace=False, tmpdir=None):
    global LAST_RESULT
    n = inputs.shape[0]
    assert n == N_TOTAL
    nc = _get_program()
    in_maps, host_terms = prep_inmaps(inputs, targets, domains, dcc_weights)
    res = run_bass_kernel_spmd(
        nc, in_maps, core_ids=list(range(N_CORES)), trace=trace, tmpdir=tmpdir
    )
    LAST_RESULT = res
    return combine(res.results, host_terms, n)


def kernel(inputs, targets, domains, dcc_weights):
    targets = np.asarray(targets).reshape(-1)
    if np.any((targets < 0) | (targets >= C)) or inputs.shape[0] != N_TOTAL:
        # IGNORE/out-of-range targets or odd shapes: exact host fallback
        # (never hit for the spec'd input distribution).
        x = np.asarray(inputs, dtype=np.float64)
        dcc = np.asarray(dcc_weights, dtype=np.float64)
        logw = np.where(dcc > 0, np.log(np.maximum(dcc, 1e-300)), -np.inf)
        scores = logw[np.asarray(domains).reshape(-1)] + x
        m = scores.max(axis=1)
        lse = m + np.log(np.exp(scores - m[:, None]).sum(axis=1))
        tgt = np.clip(targets, 0, C - 1)
        ts = scores[np.arange(x.shape[0]), tgt]
        valid = targets != IGNORE
        return np.float32(np.where(valid, lse - ts, 0.0).sum() / x.shape[0])
    return run(inputs, targets, domains, dcc_weights, trace=False)


# revision 19
# speedup vs baseline: 78.1736x; 22.2332x over previous
"""Trainium2 Bass kernel for DomainCalibratedLoss (v2).

loss = [ sum_i ln(sum_j w[d_i,j] e^{x_ij}) - sum_i (x[i,t_i] + ln w[d_i,t_i]) ] / N

Device computes the heavy term sum_i ln S_i with S_i = sum_j w[d_i,j] e^{x_ij}.
The target-score sum is an O(N) exact f64 gather on the host, and x is
delivered to the device as bf16 (validated: total rel err ~1e-4 vs the f64
reference, gate is 2e-2).

Layout/strategy (data-parallel over 8 cores, DMA-bound):
  * host sorts rows by domain and pads each domain to a multiple of T=512,
    giving domain-pure 512-row chunks; 123 chunks/core (ceil(N/T)+D-1 <= 984
    covers any domain mix). Pad rows have x=0 and are corrected exactly on
    the host.
  * x ships TRANSPOSED and group-major: xt[group, class, 4*2T] bf16, so each
    x DMA reads one contiguous 8KB run per partition (4 chunks).
  * exp runs entirely on DVE as an int16 Schraudolph exponential:
    e~ = bitcast_bf16(int16(x*(2^7*log2 e) + B16)), one fused tensor_scalar
    (mult+add) per chunk writing a separate int16 tile -- in-place aliasing
    measurably blocks the DVE fast mode (-25%). B16 is tuned so the
    resulting bias on the final loss is ~1e-4. (True exp on ACT was tried
    and removed: interleaving Exp and Ln on ACT thrashes the 2.7us
    activation-table loads.)
  * PE fuses the weight-multiply and class-reduce: S row = w_d^T @ e via two
    K=100 bf16 matmuls per chunk (lhsT = per-chunk host-filled w column).
    Chunk s of a 12-chunk round lands in psum bank s//3 at partition
    32*(s%3) (matmul out base partitions are restricted to {0,32,64}).
  * one ACT Ln per round reads S straight from PSUM [65, 4*512] and
    accum_out's each partition lane; the host sums lanes {0,32,64} per
    round column. Garbage lanes keep their garbage -- never read.

Walrus caps every engine instruction at ONE sync wait. A tiny DVE "absorber"
copy at each round start observes the Ln whose psum buffer the round reuses,
making the matmuls' psum-WAR waits transitively implied; then
_prune_redundant_waits deletes the redundant waits (asserted zero left).
"""

import numpy as np
from contextlib import ExitStack

import concourse.bass as bass
import concourse.tile as tile
from concourse import mybir
from concourse.bass_utils import run_bass_kernel_spmd

F32 = mybir.dt.float32
BF16 = mybir.dt.bfloat16
I16 = mybir.dt.int16
BF16_NP = mybir.dt.np(BF16)

N_TOTAL = 500000
N_CORES = 8
C = 200
CG = 100  # classes per contraction group
D = 8
IGNORE = 255
T = 512  # rows per chunk
CHUNKS = 123  # chunks per core (ceil(N/T) + D - 1 <= 8*123 for any domain mix)
N_PER = CHUNKS * T  # padded rows per core (65536)
ROUND = 12  # chunks per psum round (4 banks x 3 quadrant lanes)
ACT_PER_ROUND = 0  # all chunks on DVE (ACT only runs Ln; avoids exp<->ln table thrash)
DMA_GROUP = 4  # chunks fetched per x DMA (group-major DRAM: 8KB/partition runs)
XP_BUFS = 6  # x tile pool depth (DMA lookahead = XP_BUFS * DMA_GROUP chunks)
TS_INPLACE = False  # separate e tile: in-place aliasing blocks the DVE fast mode (-25% measured)
NGRP = (CHUNKS + DMA_GROUP - 1) // DMA_GROUP  # x DMA groups per core

LOG2E = float(np.log2(np.e))
A16 = float(np.float32((2 ** 7) * LOG2E))
B16_C = 7.5
B16 = float(np.float32(127 * 2 ** 7 - B16_C))


def _rounds():
    """[(round_chunks, [(col, part_hi, bank_lo, bank_hi, lanes)])] structure.

    Each round of up to 12 chunks maps chunk s -> bank s//3, lane 32*(s%3).
    Ln windows per round: one over the full banks (all 3 lanes written) and,
    for a ragged tail, one over the final partial bank.
    """
    rounds = []
    col = 0
    left = CHUNKS
    while left > 0:
        n = min(ROUND, left)
        fb, rem = n // 3, n % 3
        wins = []
        if fb:
            wins.append((col, 65, 0, fb, (0, 32, 64)))
            col += 1
        if rem:
            wins.append((col, 32 * (rem - 1) + 1, fb, fb + 1, tuple(32 * q for q in range(rem))))
            col += 1
        rounds.append((n, wins))
        left -= n
    return rounds, col


ROUNDS, N_COLS = _rounds()


def _prune_redundant_waits(nc):
    """Drop sync waits provably implied (transitively) by other waits.

    Walrus encodes at most ONE sync wait per engine instruction. Tile's
    per-proc wait emission is not transitively minimal. We compute
    happens-before vector clocks over the emitted sync graph and delete waits
    covered by (a) the same-engine predecessor's knowledge or (b) another
    wait on the same instruction.
    """
    f = nc.m.functions[0]
    insts = []
    for bb in f.blocks:
        for inst in bb.instructions:
            insts.append(inst)

    streams = {}
    pos = {}
    for inst in insts:
        eng = str(inst.engine)
        streams.setdefault(eng, []).append(inst)
        pos[inst.name] = (eng, len(streams[eng]) - 1)

    sem_updates = {}
    for inst in insts:
        si = inst.sync_info
        if si is None:
            continue
        for upd in si.on_update:
            if upd.sync_type != "semaphore" or upd.update_mode not in (
                "sem-inc",
                "sem-add-imm",
            ):
                continue
            lst = sem_updates.setdefault(upd.ant_name, [])
            prev = lst[-1][0] if lst else 0
            lst.append((prev + upd.update_value, inst.name))

    def satisfier(w):
        if w.sync_type != "semaphore" or w.wait_mode != "sem-ge-imm":
            return None
        lst = sem_updates.get(w.ant_name)
        if not lst:
            return None
        for cum, nm in lst:
            if cum >= w.wait_value:
                return nm
        return None

    vc = {nm: {} for nm in pos}

    def join(dst, src):
        changed = False
        for k, v in src.items():
            if dst.get(k, -1) < v:
                dst[k] = v
                changed = True
        return changed

    for _ in range(16):
        changed = False
        for eng, stream in streams.items():
            run = {}
            for i, inst in enumerate(stream):
                nm = inst.name
                si = inst.sync_info
                if si is not None:
                    for w in si.on_wait:
                        s = satisfier(w)
                        if s is None:
                            continue
                        sp, sidx = pos[s]
                        join(run, vc[s])
                        if run.get(sp, -1) < sidx:
                            run[sp] = sidx
                if join(vc[nm], run):
                    changed = True
                join(run, {eng: i})
        if not changed:
            break

    for eng, stream in streams.items():
        for i, inst in enumerate(stream):
            si = inst.sync_info
            if si is None or len(si.on_wait) <= 1:
                continue
            known = {}
            if i > 0:
                join(known, vc[stream[i - 1].name])
                join(known, {eng: i - 1})
            waits = list(si.on_wait)
            sats = [satisfier(w) for w in waits]
            keep = [True] * len(waits)
            for _trial in range(len(waits)):
                dropped_any = False
                for j in range(len(waits)):
                    if not keep[j] or sats[j] is None:
                        continue
                    cover = dict(known)
                    for k in range(len(waits)):
                        if k == j or not keep[k] or sats[k] is None:
                            continue
                        join(cover, vc[sats[k]])
                        skp, skidx = pos[sats[k]]
                        if cover.get(skp, -1) < skidx:
                            cover[skp] = skidx
                    sp, sidx = pos[sats[j]]
                    if cover.get(sp, -1) >= sidx:
                        keep[j] = False
                        dropped_any = True
                if not dropped_any:
                    break
            new_waits = [w for w, k in zip(waits, keep) if k]
            if len(new_waits) != len(waits):
                inst.sync_info = mybir.SyncInfo(
                    on_wait=new_waits, on_update=list(si.on_update)
                )


def build_program(num_devices=N_CORES, passes=1, act_per_round=ACT_PER_ROUND,
                  dma_group=DMA_GROUP, xp_bufs=XP_BUFS, ts_inplace=TS_INPLACE,
                  ablate=()):
    nc = bass.Bass(
        "TRN2",
        target_bir_lowering=False,
        debug=False,
        num_devices=num_devices,
    )

    ngrp = (CHUNKS + dma_group - 1) // dma_group
    xt_d = nc.dram_tensor("xt", [ngrp, CG, dma_group * 2 * T], BF16, kind="ExternalInput").ap()
    w_d = nc.dram_tensor("wall", [CG, 2 * CHUNKS], BF16, kind="ExternalInput").ap()
    out_d = nc.dram_tensor("out", [65, 16], F32, kind="ExternalOutput").ap()

    with ExitStack() as ctx:
        tc = ctx.enter_context(tile.TileContext(nc))

        singles = ctx.enter_context(tc.tile_pool(name="singles", bufs=1))
        xp = ctx.enter_context(tc.tile_pool(name="xp", bufs=xp_bufs))
        ep = ctx.enter_context(tc.tile_pool(name="ep", bufs=6))
        pp = ctx.enter_context(tc.tile_pool(name="pp", bufs=2, space="PSUM"))
        absp = ctx.enter_context(tc.tile_pool(name="absp", bufs=4))

        w_all = singles.tile([CG, 2 * CHUNKS], BF16)
        nc.sync.dma_start(out=w_all[:], in_=w_d[:, :])
        ln_scr = singles.tile([65, 4 * T], F32)
        acc = singles.tile([65, 16], F32)
        if "ln" in ablate:
            nc.vector.memset(acc[:], 0.0)
            nc.scalar.activation(ln_scr[:], ln_scr[:].bitcast(F32),
                                 mybir.ActivationFunctionType.Copy) if False else None
            nc.vector.memset(ln_scr[:], 0.0)

        ln_cols = []  # first Ln accum column per emitted round (across passes)
        for _pass in range(passes):
            c = 0
            for r, (n_chunks, wins) in enumerate(ROUNDS):
                ps = pp.tile([128, 4 * T], F32, tag="ps")
                if len(ln_cols) >= 2 and "ln" not in ablate:
                    # DVE absorber: observe the Ln whose psum buffer this
                    # round reuses, so the first matmul's psum-WAR wait is
                    # transitively implied through the DVE exp (1-wait cap).
                    ascr = absp.tile([1, 1], F32, tag="ascr")
                    nc.vector.tensor_copy(
                        ascr[:, :], acc[0:1, ln_cols[-2] : ln_cols[-2] + 1]
                    )
                for s in range(n_chunks):
                    b, q = s // 3, s % 3
                    if c % dma_group == 0:
                        ng = min(dma_group, CHUNKS - c)
                        xgrp = xp.tile([CG, dma_group * 2 * T], BF16, tag="xg")
                        if "dma" not in ablate:
                            nc.sync.dma_start(
                                out=xgrp[:, : ng * 2 * T],
                                in_=xt_d[c // dma_group, :, : ng * 2 * T],
                            )
                        else:
                            nc.sync.dma_start(
                                out=xgrp[0:1, 0:16],
                                in_=xt_d[c // dma_group, 0:1, 0:16],
                            )
                    xg = xgrp[:, (c % dma_group) * 2 * T : (c % dma_group + 1) * 2 * T]
                    is_act = s < act_per_round
                    if is_act:
                        if "exp" not in ablate:
                            nc.scalar.activation(
                                xg, xg, mybir.ActivationFunctionType.Exp
                            )
                    else:
                        if "ts" not in ablate:
                            if ts_inplace:
                                nc.vector.tensor_scalar(
                                    out=xg.bitcast(I16),
                                    in0=xg,
                                    scalar1=A16,
                                    scalar2=B16,
                                    op0=mybir.AluOpType.mult,
                                    op1=mybir.AluOpType.add,
                                )
                            else:
                                # one Schraudolph per DMA group: fewer DVE
                                # instructions, separate out tile (in-place
                                # aliasing blocks the DVE fast mode)
                                if c % dma_group == 0:
                                    egrp = ep.tile(
                                        [CG, dma_group * 2 * T], I16, tag="et"
                                    )
                                    nc.vector.tensor_scalar(
                                        out=egrp[:, : ng * 2 * T],
                                        in0=xgrp[:, : ng * 2 * T],
                                        scalar1=A16,
                                        scalar2=B16,
                                        op0=mybir.AluOpType.mult,
                                        op1=mybir.AluOpType.add,
                                    )
                                xg = egrp[
                                    :,
                                    (c % dma_group) * 2 * T : (c % dma_group + 1) * 2 * T,
                                ].bitcast(BF16)
                    if "mm" not in ablate:
                        for g in range(2):
                            nc.tensor.matmul(
                                out=ps[32 * q : 32 * q + 1, T * b : T * (b + 1)],
                                lhsT=w_all[:, 2 * c + g : 2 * c + g + 1],
                                rhs=xg[:, T * g : T * (g + 1)],
                                start=(g == 0),
                                stop=(g == 1),
                                skip_group_check=True,
                            )
                    c += 1
                if "ln" not in ablate:
                    for col, phi, blo, bhi, _lanes in wins:
                        nc.scalar.activation(
                            ln_scr[0:phi, T * blo : T * bhi],
                            ps[0:phi, T * blo : T * bhi],
                            mybir.ActivationFunctionType.Ln,
                            accum_out=acc[0:phi, col : col + 1],
                        )
                    ln_cols.append(wins[-1][0])

        nc.sync.dma_start(out=out_d[:, :], in_=acc[:])

    _prune_redundant_waits(nc)

    violations = []
    f = nc.m.functions[0]
    n_table_loads = 0
    for bb in f.blocks:
        for inst in bb.instructions:
            if type(inst).__name__ == "InstLoadActFuncSet":
                n_table_loads += 1
            si = inst.sync_info
            if si is None:
                continue
            nm = type(inst).__name__
            if nm in (
                "InstDrain",
                "InstEventSemaphore",
                "InstUnconditionalBranch",
                "InstRegisterMove",
                "InstCall",
                "InstNoOp",
            ):
                continue
            if len(si.on_wait) > 1:
                violations.append(
                    (inst.name, nm, str(inst.engine),
                     [(w.ant_name, w.wait_value) for w in si.on_wait])
                )
    nc._wait_violations = violations
    nc._n_table_loads = n_table_loads
    return nc


def prep_inmaps(inputs, targets, domains, dcc_weights, act_per_round=ACT_PER_ROUND,
                dma_group=DMA_GROUP):
    """Host prep: domain sort + pad, bf16 transpose-chunk x, per-chunk w
    columns, exact f64 target-score sum, pad corrections."""
    x = np.ascontiguousarray(np.asarray(inputs, dtype=np.float32))
    t = np.asarray(targets).astype(np.int64).reshape(-1)
    d = np.asarray(domains).astype(np.int64).reshape(-1)
    w = np.asarray(dcc_weights, dtype=np.float32)
    n = x.shape[0]

    order = np.argsort(d, kind="stable")
    counts = np.bincount(d[order], minlength=D)
    total_chunks = N_CORES * CHUNKS

    gidx = np.full(total_chunks * T, -1, np.int64)
    chunk_dom = np.zeros(total_chunks, np.int64)
    pos = 0
    src = 0
    for dd in range(D):
        cnt = int(counts[dd])
        m = (cnt + T - 1) // T
        gidx[pos : pos + cnt] = order[src : src + cnt]
        chunk_dom[pos // T : pos // T + m] = dd
        pos += m * T
        src += cnt
    assert pos // T <= total_chunks

    w_bf = w.astype(BF16_NP)
    w_bf_f = w_bf.astype(np.float32)

    s_in_round = np.arange(CHUNKS) % ROUND
    is_act_chunk = s_in_round < act_per_round  # same pattern every core

    # pad correction (exact): pad rows have x=0
    e_pad_act = 1.0
    e_pad_dve = float(
        np.round(np.float32(0) * np.float32(A16) + np.float32(B16))
        .astype(np.int16).view(BF16_NP)
    )
    pad_corr = 0.0
    npads = np.bincount(
        np.nonzero(gidx < 0)[0] // T, minlength=total_chunks
    )
    for cidx in np.nonzero(npads)[0]:
        ep = e_pad_act if is_act_chunk[cidx % CHUNKS] else e_pad_dve
        s_pad = float((w_bf_f[chunk_dom[cidx]] * np.float32(ep)).sum(dtype=np.float32))
        pad_corr += float(npads[cidx]) * float(np.log(s_pad))

    # exact host-side target-score sum (f64, from the original f32 x)
    logw = np.log(w.astype(np.float64))
    tc_ = np.clip(t, 0, C - 1)
    valid = t != IGNORE
    t_sum = float(
        np.where(valid, x[np.arange(n), tc_].astype(np.float64) + logw[d, tc_], 0.0).sum()
    )

    in_maps = []
    for core in range(N_CORES):
        sl = slice(core * N_PER, (core + 1) * N_PER)
        gi = gidx[sl]
        xb = np.zeros((N_PER, C), BF16_NP)
        v = gi >= 0
        xb[v] = x[gi[v]].astype(BF16_NP)
        xtc = xb.reshape(CHUNKS, T, 2, CG).transpose(0, 3, 2, 1).reshape(
            CHUNKS, CG, 2 * T
        )
        ngrp = (CHUNKS + dma_group - 1) // dma_group
        pad_slots = ngrp * dma_group - CHUNKS
        if pad_slots:
            xtc = np.concatenate(
                [xtc, np.zeros((pad_slots, CG, 2 * T), BF16_NP)], axis=0
            )
        xt = np.ascontiguousarray(
            xtc.reshape(ngrp, dma_group, CG, 2 * T).transpose(0, 2, 1, 3)
        ).reshape(ngrp, CG, dma_group * 2 * T)
        doms = chunk_dom[core * CHUNKS : (core + 1) * CHUNKS]
        wall = np.empty((CG, 2 * CHUNKS), BF16_NP)
        wall[:, 0::2] = w_bf[doms, 0:CG].T
        wall[:, 1::2] = w_bf[doms, CG:C].T
        in_maps.append({"xt": xt, "wall": wall})

    host_terms = (pad_corr, t_sum)
    return in_maps, host_terms


def combine(results, host_terms, n_total):
    pad_corr, t_sum = host_terms
    total_ln = 0.0
    for r in results:
        o = np.asarray(r["out"], dtype=np.float64)
        for _n_chunks, wins in ROUNDS:
            for col, _phi, _blo, _bhi, lanes in wins:
                total_ln += o[list(lanes), col].sum()
    return np.float32((total_ln - pad_corr - t_sum) / n_total)


_PROGRAM_CACHE = {}


def _get_program():
    key = "v2"
    if key not in _PROGRAM_CACHE:
        _PROGRAM_CACHE[key] = build_program()
    return _PROGRAM_CACHE[key]


LAST_RESULT = None


def run(inputs, targets, domains, dcc_weights, trace=False, tmpdir=None):
    global LAST_RESULT
    n = inputs.shape[0]
    assert n == N_TOTAL
    nc = _get_program()
    in_maps, host_terms = prep_inmaps(inputs, targets, domains, dcc_weights)
    res = run_bass_kernel_spmd(
        nc, in_maps, core_ids=list(range(N_CORES)), trace=trace, tmpdir=tmpdir
    )
    LAST_RESULT = res
    return combine(res.results, host_terms, n)


def kernel(inputs, targets, domains, dcc_weights):
    targets = np.asarray(targets).reshape(-1)
    if np.any((targets < 0) | (targets >= C)) or inputs.shape[0] != N_TOTAL:
        # IGNORE/out-of-range targets or odd shapes: exact host fallback
        # (never hit for the spec'd input distribution).
        x = np.asarray(inputs, dtype=np.float64)
        dcc = np.asarray(dcc_weights, dtype=np.float64)
        logw = np.where(dcc > 0, np.log(np.maximum(dcc, 1e-300)), -np.inf)
        scores = logw[np.asarray(domains).reshape(-1)] + x
        m = scores.max(axis=1)
        lse = m + np.log(np.exp(scores - m[:, None]).sum(axis=1))
        tgt = np.clip(targets, 0, C - 1)
        ts = scores[np.arange(x.shape[0]), tgt]
        valid = targets != IGNORE
        return np.float32(np.where(valid, lse - ts, 0.0).sum() / x.shape[0])
    return run(inputs, targets, domains, dcc_weights, trace=False)
